# revision 16
# baseline (speedup 1.0000x reference)
"""Trainium2 Bass kernel: depthwise (per-sample, per-channel) 15x15 'same'
true convolution of 1024x3 images of 128x128, data-parallel over 8 NeuronCores.

Stacked-block formulation. out[y,x] = sum_{dy,dx} Xpad[y+dy,x+dx]*Wf[dy,dx].
MAIN (y 0..99): two 50-row y-blocks stacked in one K=128 contraction:
  stationary TS [128, 100] block-diagonal, rows 0..63 cols 0..49 = T_rel,
  rows 64..127 cols 50..99 = T_rel, where T_rel[i,j] = Wf[i-j,dx] (64x50 band).
  Moving operand X2 [128, x]: partitions 0..63 = Xpad rows 0..63, partitions
  64..127 = Xpad rows 50..113; one N=128 stream covers 100 output rows per dx.
LEFTOVER (y 100..127): 5 matmuls each packing 3 dx values: stationary
  T28E [126, 84] block-diagonal over 42-row groups c=0..2 (dx = 3t+c), moving
  XE [126, x] = rows 100..141 replicated 3x with shift c baked in; PSUM
  accumulates over t; the host sums the 3 column slabs.
PE cost: (15+5) matmuls x 128 streamed rows = 2560 cycles/image (vs 3840 for
the 2-block classic/flipped duals). All loads (M=100/84) hide under streams.
DMAs batched 8 images with packet-major layouts; zeros in block-diagonal
stationaries are persistent (memset once per buffer).
"""
import sys

sys.path.insert(0, "/opt/trn_rl_repo")

import numpy as np

_N_CORES = 8
_BN, _C, _P, _K = 1024, 3, 128, 15
_PAIRS_PER_CORE = (_BN // _N_CORES) * _C  # 384
_GROUP = 8

_nc_cache = {}


def _build_nc(n_pairs: int, bufs: int = 3, psum_bufs: int = 4):
    import concourse.bacc as bacc
    import concourse.mybir as mybir
    from concourse import tile
    from concourse.ap import AP

    FP16 = mybir.dt.float16
    FP32 = mybir.dt.float32
    G = _GROUP
    n_groups = n_pairs // G

    nc = bacc.Bacc("TRN2", target_bir_lowering=False, debug=False)
    xpad_d = nc.dram_tensor("xpad", [142, n_pairs, 142], FP16, kind="ExternalInput")
    # T_rel [64, img, 50*15] (j-major, dx-minor)
    trel_d = nc.dram_tensor("trel", [64, n_pairs, 750], FP16, kind="ExternalInput")
    # T28E block-diag [126, img, 84*5] (m-major, t-minor)
    t28_d = nc.dram_tensor("t28", [126, n_pairs, 420], FP16, kind="ExternalInput")
    # XE edge rows, shifts baked [126, img, 142]
    xe_d = nc.dram_tensor("xe", [126, n_pairs, 142], FP16, kind="ExternalInput")
    outa_d = nc.dram_tensor("outa", [100, n_pairs, 128], FP16, kind="ExternalOutput")
    outb_d = nc.dram_tensor("outb", [84, n_pairs, 128], FP16, kind="ExternalOutput")

    TS_EXT = 1500  # per-image elems/partition in ts: [m(100), dx(15)]
    TE_EXT = 420   # per-image elems/partition in te: [m(84), t(5)]

    with tile.TileContext(nc) as tc:
        with (
            tc.tile_pool(name="x2", bufs=bufs) as x2_pool,
            tc.tile_pool(name="xe", bufs=bufs) as xe_pool,
            tc.tile_pool(name="ts", bufs=bufs) as ts_pool,
            tc.tile_pool(name="te", bufs=bufs) as te_pool,
            tc.tile_pool(name="ot", bufs=bufs) as ot_pool,
            tc.tile_pool(name="ps", bufs=psum_bufs, space="PSUM") as ps_pool,
        ):
            for g in range(n_groups):
                s = slice(g * G, (g + 1) * G)
                x2 = x2_pool.tile([128, G * 142], FP16, tag="x2")
                xe = xe_pool.tile([126, G * 142], FP16, tag="xe")
                ts = ts_pool.tile([128, G * TS_EXT], FP16, tag="ts")
                te = te_pool.tile([126, G * TE_EXT], FP16, tag="te")
                ota = ot_pool.tile([100, G * 128], FP16, tag="ota")
                otb = ot_pool.tile([84, G * 128], FP16, tag="otb")
                if g < bufs:
                    # first use of each rotating buffer: zero the
                    # block-diagonal exteriors once; they stay zero forever
                    nc.vector.memset(ts[:], 0.0)
                    nc.vector.memset(te[:], 0.0)

                # X main: rows 0..63 -> partitions 0..63; rows 50..113 -> 64..127
                nc.sync.dma_start(out=x2[0:64, :], in_=xpad_d[0:64, s, :])
                nc.sync.dma_start(out=x2[64:128, :], in_=xpad_d[50:114, s, :])
                nc.sync.dma_start(out=xe[:], in_=xe_d[:, s, :])
                # TS region A: partitions 0..63, per-image elems [0, 750)
                in_a = trel_d[:, s, :]
                out_a = AP(tensor=ts[:].tensor, offset=ts[:].offset,
                           ap=[[G * TS_EXT, 64], [TS_EXT, G], [1, 750]])
                nc.scalar.dma_start(out=out_a, in_=in_a)
                # TS region B: partitions 64..127, elems [750, 1500)
                out_b = AP(tensor=ts[:].tensor,
                           offset=ts[:].offset + 64 * G * TS_EXT + 750,
                           ap=[[G * TS_EXT, 64], [TS_EXT, G], [1, 750]])
                nc.scalar.dma_start(out=out_b, in_=in_a)
                nc.scalar.dma_start(out=te[:], in_=t28_d[:, s, :])

                for j in range(G):
                    psm = ps_pool.tile([100, 128], FP32, tag="psm")
                    psl = ps_pool.tile([84, 128], FP32, tag="psl")
                    xo = j * 142
                    base = ts[:]
                    for dx in range(15):
                        lhsT = AP(tensor=base.tensor,
                                  offset=base.offset + j * TS_EXT + dx,
                                  ap=[[G * TS_EXT, 128], [15, 100]])
                        nc.tensor.matmul(
                            psm[:], lhsT, x2[:, xo + dx : xo + dx + 128],
                            start=(dx == 0), stop=(dx == 14),
                        )
                    baset = te[:]
                    for t in range(5):
                        lhsT = AP(tensor=baset.tensor,
                                  offset=baset.offset + j * TE_EXT + t,
                                  ap=[[G * TE_EXT, 126], [5, 84]])
                        nc.tensor.matmul(
                            psl[:], lhsT, xe[:, xo + 3 * t : xo + 3 * t + 128],
                            start=(t == 0), stop=(t == 4),
                        )
                    nc.vector.tensor_copy(ota[:, j * 128 : (j + 1) * 128], psm[:])
                    nc.scalar.copy(otb[:, j * 128 : (j + 1) * 128], psl[:])
                nc.sync.dma_start(out=outa_d[:, s, :], in_=ota[:])
                nc.scalar.dma_start(out=outb_d[:, s, :], in_=otb[:])

    nc.compile()
    return nc


def _host_prep(patches_pairs: np.ndarray, kernels_pairs: np.ndarray):
    NP = patches_pairs.shape[0]
    Xp = np.zeros((NP, 142, 142), dtype=np.float16)
    Xp[:, 7:135, 7:135] = patches_pairs.astype(np.float16)
    W = kernels_pairs[:, ::-1, ::-1].astype(np.float16)  # Wf [NP, 15, 15]

    # T_rel[i, j, dx] = Wf[i-j, dx], i in 0..63, j in 0..49 (band 0<=i-j<15)
    H = np.zeros((NP, 127, 15), dtype=np.float16)
    H[:, 49:64, :] = W
    s0, s1, s2 = H.strides
    A = np.lib.stride_tricks.as_strided(
        H[:, 49:, :], shape=(NP, 64, 50, 15), strides=(s0, s1, -s1, s2)
    )  # [NP, i, j, dx]
    trel = np.ascontiguousarray(
        A.transpose(1, 0, 2, 3).reshape(64, NP, 750)
    )

    # T28[i', j, dx] = Wf[i'-j, dx], i' in 0..41, j in 0..27
    H2 = np.zeros((NP, 69, 15), dtype=np.float16)
    H2[:, 27:42, :] = W
    z0, z1, z2 = H2.strides
    A2 = np.lib.stride_tricks.as_strided(
        H2[:, 27:, :], shape=(NP, 42, 28, 15), strides=(z0, z1, -z1, z2)
    )  # [NP, i', j, dx]
    # T28E[k=42c+i', p, m=28c+j, t] = T28[i', j, dx=3t+c]; block-diag in (c)
    t28e = np.zeros((126, NP, 84, 5), dtype=np.float16)
    for c in range(3):
        t28e[42 * c : 42 * (c + 1), :, 28 * c : 28 * (c + 1), :] = (
            A2[:, :, :, c::3].transpose(1, 0, 2, 3)
        )
    t28e = np.ascontiguousarray(t28e.reshape(126, NP, 420))

    # XE[k=42c+i', p, x] = Xpad[p, 100+i', x+c]  (shift c baked, clipped)
    xe = np.zeros((126, NP, 142), dtype=np.float16)
    for c in range(3):
        xe[42 * c : 42 * (c + 1), :, : 142 - c] = (
            Xp[:, 100:142, c:].transpose(1, 0, 2)
        )
    xe = np.ascontiguousarray(xe)

    Xp = np.ascontiguousarray(Xp.transpose(1, 0, 2))
    return Xp, trel, t28e, xe


def kernel(patches, kernels, kernel_size, patch_size, fft_size, _collect_results=None):
    """Full inputs in, full output out. Shards BN across 8 cores."""
    from concourse.bass_utils import run_bass_kernel_spmd

    patches = np.asarray(patches)
    kernels = np.asarray(kernels)
    assert patches.shape == (_BN, _C, _P, _P), patches.shape
    assert kernels.shape == (_BN, _C, _K, _K), kernels.shape

    if "nc" not in _nc_cache:
        _nc_cache["nc"] = _build_nc(_PAIRS_PER_CORE)
    nc = _nc_cache["nc"]

    bn_per_core = _BN // _N_CORES
    in_maps = []
    for core in range(_N_CORES):
        sl = slice(core * bn_per_core, (core + 1) * bn_per_core)
        pp = patches[sl].reshape(-1, _P, _P)
        kp = kernels[sl].reshape(-1, _K, _K)
        xpad, trel, t28e, xe = _host_prep(pp, kp)
        in_maps.append({"xpad": xpad, "trel": trel, "t28": t28e, "xe": xe})

    res = run_bass_kernel_spmd(nc, in_maps, core_ids=list(range(_N_CORES)))
    if _collect_results is not None:
        _collect_results.append(res)

    out = np.empty((_BN, _C, _P, _P), dtype=np.float32)
    for core in range(_N_CORES):
        sl = slice(core * bn_per_core, (core + 1) * bn_per_core)
        oa = res.results[core]["outa"].astype(np.float32)  # [100, NP, 128]
        ob = res.results[core]["outb"].astype(np.float32)  # [84, NP, 128]
        top = oa.transpose(1, 0, 2)                        # [NP, 100, 128]
        bot = ob.reshape(3, 28, -1, 128).sum(axis=0).transpose(1, 0, 2)
        full = np.concatenate([top, bot], axis=1)          # [NP, 128, 128]
        out[sl] = full.reshape(bn_per_core, _C, _P, _P)
    return out


# revision 18
# speedup vs baseline: 2.1157x; 2.1157x over previous
"""Trainium2 Bass kernel: depthwise (per-sample, per-channel) 15x15 'same'
true convolution of 1024x3 images of 128x128, data-parallel over 8 NeuronCores.

Flipped-operand formulation (per (bn,c) pair, P=128, K=15, pad=7):
    out[y, x] = sum_{dy,dx} Xpad[y+dy, x+dx] * W[dy,dx],  W = flip(kernel)
y is split into 2 blocks of 64 rows. For block y0 and each dx:
    outT[x, y0+j] += sum_i Xpad[y0+i, x+dx] * T[i, dx, j]
with the banded block-Toeplitz T[i, dx, j] = W[i-j, dx] (0 <= i-j < 15).
The stationary operand is the padded-image slice Xpad[y0:y0+78, dx:dx+128]
(plain AP view); the moving operand is the compact Toeplitz streaming N=64
columns. PSUM holds the transposed output [x=128, y], accumulated over dx.

DMA economics dominate this kernel: DGE descriptor generation costs ~7.5ns
per packet on the issuing queue, so inputs/outputs are batched 8 images per
DMA with packet-major DRAM layouts ([row, img, col] etc.) giving one long
contiguous run per partition per group. X/out DMAs issue on the sync queue,
toep on the scalar (Activation) queue. Output fp16, transposed; the host
transposes back and upcasts. Sharding: pure data parallel over BN (384
independent images per core).
"""
import sys

sys.path.insert(0, "/opt/trn_rl_repo")

import numpy as np

_N_CORES = 8
_BN, _C, _P, _K = 1024, 3, 128, 15
_PAIRS_PER_CORE = (_BN // _N_CORES) * _C  # 384
_GROUP = 8

_nc_cache = {}


def _build_nc(n_pairs: int, bufs: int = 3, psum_bufs: int = 4):
    import concourse.bacc as bacc
    import concourse.mybir as mybir
    from concourse import tile

    FP16 = mybir.dt.float16
    FP32 = mybir.dt.float32
    G = _GROUP
    n_groups = n_pairs // G

    nc = bacc.Bacc("TRN2", target_bir_lowering=False, debug=False)
    xpad_d = nc.dram_tensor("xpad", [142, n_pairs, 142], FP16, kind="ExternalInput")
    toep_d = nc.dram_tensor("toep", [78, n_pairs, 960], FP16, kind="ExternalInput")
    out_d = nc.dram_tensor("out", [128, n_pairs, 128], FP16, kind="ExternalOutput")

    with tile.TileContext(nc) as tc:
        with (
            tc.tile_pool(name="xa", bufs=bufs) as xa_pool,
            tc.tile_pool(name="xb", bufs=bufs) as xb_pool,
            tc.tile_pool(name="tt", bufs=bufs) as tt_pool,
            tc.tile_pool(name="ot", bufs=bufs) as ot_pool,
            tc.tile_pool(name="ps", bufs=psum_bufs, space="PSUM") as ps_pool,
        ):
            for g in range(n_groups):
                s = slice(g * G, (g + 1) * G)
                xa = xa_pool.tile([78, G * 142], FP16, tag="xa")
                xb = xb_pool.tile([78, G * 142], FP16, tag="xb")
                tt = tt_pool.tile([78, G * 960], FP16, tag="tt")
                ot = ot_pool.tile([128, G * 128], FP16, tag="ot")
                if g == 0:
                    # land image 0's slices first to cut pipeline warmup
                    s0 = slice(0, 1)
                    s1 = slice(1, G)
                    nc.sync.dma_start(out=xa[:, 0:142], in_=xpad_d[0:78, s0, :])
                    nc.scalar.dma_start(out=tt[:, 0:960], in_=toep_d[:, s0, :])
                    nc.sync.dma_start(out=xb[:, 0:142], in_=xpad_d[64:142, s0, :])
                    nc.sync.dma_start(out=xa[:, 142:], in_=xpad_d[0:78, s1, :])
                    nc.sync.dma_start(out=xb[:, 142:], in_=xpad_d[64:142, s1, :])
                    nc.scalar.dma_start(out=tt[:, 960:], in_=toep_d[:, s1, :])
                else:
                    nc.sync.dma_start(out=xa[:], in_=xpad_d[0:78, s, :])
                    nc.sync.dma_start(out=xb[:], in_=xpad_d[64:142, s, :])
                    nc.scalar.dma_start(out=tt[:], in_=toep_d[:, s, :])

                for j in range(G):
                    ps0 = ps_pool.tile([128, 64], FP32, tag="ps0")
                    ps1 = ps_pool.tile([128, 64], FP32, tag="ps1")
                    xo = j * 142
                    to = j * 960
                    for dx in range(15):
                        tslc = tt[:, to + dx * 64 : to + (dx + 1) * 64]
                        nc.tensor.matmul(
                            ps0[:], xa[:, xo + dx : xo + dx + 128], tslc,
                            start=(dx == 0), stop=(dx == 14),
                        )
                        nc.tensor.matmul(
                            ps1[:], xb[:, xo + dx : xo + dx + 128], tslc,
                            start=(dx == 0), stop=(dx == 14),
                        )
                    oo = j * 128
                    nc.vector.tensor_copy(ot[:, oo : oo + 64], ps0[:])
                    nc.scalar.copy(ot[:, oo + 64 : oo + 128], ps1[:])
                nc.scalar.dma_start(out=out_d[:, s, :], in_=ot[:])

    nc.compile()
    return nc


def _host_prep(patches_pairs: np.ndarray, kernels_pairs: np.ndarray):
    """[NP,128,128] f32, [NP,15,15] f32 -> xpad [142,NP,142] fp16 (row-major
    over images), toep [78,NP,960] fp16 with toep[i,p,dx*64+j] =
    flip(kern)[i-j, dx]."""
    NP = patches_pairs.shape[0]
    Xp = np.zeros((NP, 142, 142), dtype=np.float16)
    Xp[:, 7:135, 7:135] = patches_pairs.astype(np.float16)
    Xp = np.ascontiguousarray(Xp.transpose(1, 0, 2))
    W = kernels_pairs[:, ::-1, ::-1].astype(np.float16)
    H = np.zeros((NP, 141, 15), dtype=np.float16)
    H[:, 63:78, :] = W
    s0, s1, s2 = H.strides
    A = np.lib.stride_tricks.as_strided(
        H[:, 63:, :], shape=(NP, 78, 64, 15), strides=(s0, s1, -s1, s2)
    )
    T = np.ascontiguousarray(
        A.transpose(1, 0, 3, 2).reshape(78, NP, 960)
    )
    return Xp, T


def kernel(patches, kernels, kernel_size, patch_size, fft_size, _collect_results=None):
    """Full inputs in, full output out. Shards BN across 8 cores."""
    from concourse.bass_utils import run_bass_kernel_spmd

    patches = np.asarray(patches)
    kernels = np.asarray(kernels)
    assert patches.shape == (_BN, _C, _P, _P), patches.shape
    assert kernels.shape == (_BN, _C, _K, _K), kernels.shape

    if "nc" not in _nc_cache:
        _nc_cache["nc"] = _build_nc(_PAIRS_PER_CORE)
    nc = _nc_cache["nc"]

    bn_per_core = _BN // _N_CORES
    in_maps = []
    for core in range(_N_CORES):
        sl = slice(core * bn_per_core, (core + 1) * bn_per_core)
        pp = patches[sl].reshape(-1, _P, _P)
        kp = kernels[sl].reshape(-1, _K, _K)
        xpad, toep = _host_prep(pp, kp)
        in_maps.append({"xpad": xpad, "toep": toep})

    res = run_bass_kernel_spmd(nc, in_maps, core_ids=list(range(_N_CORES)))
    if _collect_results is not None:
        _collect_results.append(res)

    out = np.empty((_BN, _C, _P, _P), dtype=np.float32)
    for core in range(_N_CORES):
        sl = slice(core * bn_per_core, (core + 1) * bn_per_core)
        # device emits outT [x, pair, y] -> [pair, y, x]
        outT = res.results[core]["out"].astype(np.float32)
        out[sl] = outT.transpose(1, 2, 0).reshape(bn_per_core, _C, _P, _P)
    return out


# revision 19
# speedup vs baseline: 2.1336x; 1.0085x over previous
"""Trainium2 Bass kernel: depthwise (per-sample, per-channel) 15x15 'same'
true convolution of 1024x3 images of 128x128, data-parallel over 8 NeuronCores.

Flipped-operand formulation (per (bn,c) pair, P=128, K=15, pad=7):
    out[y, x] = sum_{dy,dx} Xpad[y+dy, x+dx] * W[dy,dx],  W = flip(kernel)
y is split into 2 blocks of 64 rows. For block y0 and each dx:
    outT[x, y0+j] += sum_i Xpad[y0+i, x+dx] * T[i, dx, j]
with the banded block-Toeplitz T[i, dx, j] = W[i-j, dx] (0 <= i-j < 15).
The stationary operand is the padded-image slice Xpad[y0:y0+78, dx:dx+128]
(plain AP view); the moving operand is the compact Toeplitz streaming N=64
columns. PSUM holds the transposed output [x=128, y], accumulated over dx.

DMA economics dominate this kernel: DGE descriptor generation costs ~7.5ns
per packet on the issuing queue, so inputs/outputs are batched 8 images per
DMA with packet-major DRAM layouts ([row, img, col] etc.) giving one long
contiguous run per partition per group. X/out DMAs issue on the sync queue,
toep on the scalar (Activation) queue. Output fp16, transposed; the host
transposes back and upcasts. Sharding: pure data parallel over BN (384
independent images per core).
"""
import sys

sys.path.insert(0, "/opt/trn_rl_repo")

import numpy as np

_N_CORES = 8
_BN, _C, _P, _K = 1024, 3, 128, 15
_PAIRS_PER_CORE = (_BN // _N_CORES) * _C  # 384
_GROUP = 8

_nc_cache = {}


def _build_nc(n_pairs: int, bufs: int = 3, psum_bufs: int = 4):
    import concourse.bacc as bacc
    import concourse.mybir as mybir
    from concourse import tile

    FP16 = mybir.dt.float16
    FP32 = mybir.dt.float32
    G = _GROUP
    n_groups = n_pairs // G

    nc = bacc.Bacc("TRN2", target_bir_lowering=False, debug=False)
    xpad_d = nc.dram_tensor("xpad", [142, n_pairs, 142], FP16, kind="ExternalInput")
    toep_d = nc.dram_tensor("toep", [78, n_pairs, 960], FP16, kind="ExternalInput")
    out_d = nc.dram_tensor("out", [128, n_pairs, 128], FP16, kind="ExternalOutput")

    with tile.TileContext(nc) as tc:
        with (
            tc.tile_pool(name="xa", bufs=bufs) as xa_pool,
            tc.tile_pool(name="xb", bufs=bufs) as xb_pool,
            tc.tile_pool(name="tt", bufs=bufs) as tt_pool,
            tc.tile_pool(name="ot", bufs=bufs) as ot_pool,
            tc.tile_pool(name="ps", bufs=psum_bufs, space="PSUM") as ps_pool,
        ):
            for g in range(n_groups):
                s = slice(g * G, (g + 1) * G)
                xa = xa_pool.tile([78, G * 142], FP16, tag="xa")
                xb = xb_pool.tile([78, G * 142], FP16, tag="xb")
                tt = tt_pool.tile([78, G * 960], FP16, tag="tt")
                ot = ot_pool.tile([128, G * 128], FP16, tag="ot")
                if g == 0:
                    # land image 0's slices first to cut pipeline warmup
                    s0 = slice(0, 1)
                    s1 = slice(1, G)
                    nc.sync.dma_start(out=xa[:, 0:142], in_=xpad_d[0:78, s0, :])
                    nc.scalar.dma_start(out=tt[:, 0:960], in_=toep_d[:, s0, :])
                    nc.sync.dma_start(out=xb[:, 0:142], in_=xpad_d[64:142, s0, :])
                    nc.sync.dma_start(out=xa[:, 142:], in_=xpad_d[0:78, s1, :])
                    nc.sync.dma_start(out=xb[:, 142:], in_=xpad_d[64:142, s1, :])
                    nc.scalar.dma_start(out=tt[:, 960:], in_=toep_d[:, s1, :])
                else:
                    nc.sync.dma_start(out=xa[:], in_=xpad_d[0:78, s, :])
                    nc.sync.dma_start(out=xb[:], in_=xpad_d[64:142, s, :])
                    nc.scalar.dma_start(out=tt[:], in_=toep_d[:, s, :])

                for j in range(G):
                    ps0 = ps_pool.tile([128, 64], FP32, tag="ps0")
                    ps1 = ps_pool.tile([128, 64], FP32, tag="ps1")
                    xo = j * 142
                    to = j * 960
                    for dx in range(15):
                        nc.tensor.matmul(
                            ps0[:], xa[:, xo + dx : xo + dx + 128],
                            tt[:, to + dx * 64 : to + (dx + 1) * 64],
                            start=(dx == 0), stop=(dx == 14),
                        )
                    oo = j * 128
                    nc.vector.tensor_copy(ot[:, oo : oo + 64], ps0[:])
                    for dx in range(15):
                        nc.tensor.matmul(
                            ps1[:], xb[:, xo + dx : xo + dx + 128],
                            tt[:, to + dx * 64 : to + (dx + 1) * 64],
                            start=(dx == 0), stop=(dx == 14),
                        )
                    nc.vector.tensor_copy(ot[:, oo + 64 : oo + 128], ps1[:])
                nc.scalar.dma_start(out=out_d[:, s, :], in_=ot[:])

    nc.compile()
    return nc


def _host_prep(patches_pairs: np.ndarray, kernels_pairs: np.ndarray):
    """[NP,128,128] f32, [NP,15,15] f32 -> xpad [142,NP,142] fp16 (row-major
    over images), toep [78,NP,960] fp16 with toep[i,p,dx*64+j] =
    flip(kern)[i-j, dx]."""
    NP = patches_pairs.shape[0]
    Xp = np.zeros((NP, 142, 142), dtype=np.float16)
    Xp[:, 7:135, 7:135] = patches_pairs.astype(np.float16)
    Xp = np.ascontiguousarray(Xp.transpose(1, 0, 2))
    W = kernels_pairs[:, ::-1, ::-1].astype(np.float16)
    H = np.zeros((NP, 141, 15), dtype=np.float16)
    H[:, 63:78, :] = W
    s0, s1, s2 = H.strides
    A = np.lib.stride_tricks.as_strided(
        H[:, 63:, :], shape=(NP, 78, 64, 15), strides=(s0, s1, -s1, s2)
    )
    T = np.ascontiguousarray(
        A.transpose(1, 0, 3, 2).reshape(78, NP, 960)
    )
    return Xp, T


def kernel(patches, kernels, kernel_size, patch_size, fft_size, _collect_results=None):
    """Full inputs in, full output out. Shards BN across 8 cores."""
    from concourse.bass_utils import run_bass_kernel_spmd

    patches = np.asarray(patches)
    kernels = np.asarray(kernels)
    assert patches.shape == (_BN, _C, _P, _P), patches.shape
    assert kernels.shape == (_BN, _C, _K, _K), kernels.shape

    if "nc" not in _nc_cache:
        _nc_cache["nc"] = _build_nc(_PAIRS_PER_CORE)
    nc = _nc_cache["nc"]

    bn_per_core = _BN // _N_CORES
    in_maps = []
    for core in range(_N_CORES):
        sl = slice(core * bn_per_core, (core + 1) * bn_per_core)
        pp = patches[sl].reshape(-1, _P, _P)
        kp = kernels[sl].reshape(-1, _K, _K)
        xpad, toep = _host_prep(pp, kp)
        in_maps.append({"xpad": xpad, "toep": toep})

    res = run_bass_kernel_spmd(nc, in_maps, core_ids=list(range(_N_CORES)))
    if _collect_results is not None:
        _collect_results.append(res)

    out = np.empty((_BN, _C, _P, _P), dtype=np.float32)
    for core in range(_N_CORES):
        sl = slice(core * bn_per_core, (core + 1) * bn_per_core)
        # device emits outT [x, pair, y] -> [pair, y, x]
        outT = res.results[core]["out"].astype(np.float32)
        out[sl] = outT.transpose(1, 2, 0).reshape(bn_per_core, _C, _P, _P)
    return out
